# revision 1
# baseline (speedup 1.0000x reference)
"""CrossLayerTranscoder Trainium2 kernel, 8-core feature-parallel.

Sharding: dict dim (4096) split 512/core. Encode computes pre^T[f,b] slices
in fp32 (PE), relu+bias fused in the PSUM evacuation (ACT). Per-layer exact
global top-64: PE-transpose pre^T into [b,f] tiles, AllToAll the pre slices
so each core holds full 4096-wide rows for its 256-row shard, then one
8-round max8+match_replace select per row-tile (DVE) yields the exact
64th-largest threshold; AllGather thresholds and mask pre^T in [f,b] space
(acts stored bf16). Triangular decode recon^T[j] = sum_{i<=j} W_dec[i,j]^T
acts^T in bf16 (1 PE cycle/row, fp32 PSUM accumulate) with the full i-chain
accumulated in 6 PSUM banks per 512-row batch chunk, then per-j
ReduceScatter of the 8 partial sums; each core returns its 96-row o-shard
of recon^T and the host concatenates and transposes.
"""
import os
from contextlib import ExitStack

import numpy as np

L = 12          # layers
B = 2048        # batch rows
D = 768         # d_in
FD = 4096       # dict size
OD = 768        # d_out
TOPK = 64
NCORE = 8
FC = FD // NCORE            # 512 local features
BCH = 512                   # matmul moving-dim chunk
NB = B // BCH               # 4
NBT = B // 128              # 16 topk row tiles
KD = D // 128               # 6 encode k-tiles
NFT = FC // 128             # 4 local f-tiles
NOT = OD // 128             # 6 o-tiles
OSH = OD // NCORE           # 96 output rows per core
BSH = B // NCORE            # 256 threshold rows per core
NEG = -3.0e38
PAIRS = [(i, j) for j in range(L) for i in range(j + 1)]   # 78, j-major


def _build_nc(sim=False, no_decode=False, topk_rounds=8, no_encode=False):
    """sim=True: single-core, collectives stripped (TimelineSim timing)."""
    import concourse.bacc as bacc
    import concourse.mybir as mybir
    import concourse.tile as tile

    F32 = mybir.dt.float32
    RELU = mybir.ActivationFunctionType.Relu
    GE = mybir.AluOpType.is_ge
    MUL = mybir.AluOpType.mult
    ADD = mybir.AluOpType.add
    BYP = mybir.AluOpType.bypass
    RG = [list(range(NCORE))]

    nc = bacc.Bacc("TRN2", target_bir_lowering=False, debug=False,
                   num_devices=1 if sim else NCORE)

    x_d = nc.dram_tensor("x_t", [L, D, B], F32, kind="ExternalInput").ap()
    we_d = nc.dram_tensor("w_enc_sl", [L, D, FC], F32, kind="ExternalInput").ap()
    be_d = nc.dram_tensor("b_enc_sl", [L, FC], F32, kind="ExternalInput").ap()
    BF16 = mybir.dt.bfloat16
    wd_d = nc.dram_tensor("w_dec_sl", [len(PAIRS), 128, NFT * OD], BF16,
                          kind="ExternalInput").ap()
    bd_d = nc.dram_tensor("b_dec_sh", [L, OSH], F32, kind="ExternalInput").ap()
    id_d = nc.dram_tensor("ident", [128, 128], F32, kind="ExternalInput").ap()
    out_d = nc.dram_tensor("out_shard", [L, OSH, B], F32,
                           kind="ExternalOutput").ap()

    with tile.TileContext(nc) as tc, ExitStack() as ctx:
        sb_const = ctx.enter_context(tc.tile_pool(name="const", bufs=1))
        sb_x = ctx.enter_context(tc.tile_pool(name="xt", bufs=7))
        sb_we = ctx.enter_context(tc.tile_pool(name="we", bufs=6))
        sb_be = ctx.enter_context(tc.tile_pool(name="be", bufs=8))
        sb_pre = ctx.enter_context(tc.tile_pool(name="pre", bufs=8))
        sb_bf = ctx.enter_context(tc.tile_pool(name="prebf", bufs=2))
        sb_sel = ctx.enter_context(tc.tile_pool(name="sel", bufs=2))
        sb_t = ctx.enter_context(tc.tile_pool(name="tsel", bufs=4))
        sb_tb = ctx.enter_context(tc.tile_pool(name="tbc", bufs=1))
        sb_msk = ctx.enter_context(tc.tile_pool(name="msk", bufs=1))
        sb_wd = ctx.enter_context(tc.tile_pool(name="wd", bufs=4))
        sb_ad = ctx.enter_context(tc.tile_pool(name="ad", bufs=3))
        sb_ev = ctx.enter_context(tc.tile_pool(name="ev", bufs=3))
        sb_out = ctx.enter_context(tc.tile_pool(name="outp", bufs=1))
        sb_bd = ctx.enter_context(tc.tile_pool(name="bdec", bufs=2))

        ps_enc = ctx.enter_context(tc.tile_pool(name="psenc", bufs=2,
                                                space="PSUM"))
        ps_dec = ctx.enter_context(tc.tile_pool(name="psdec", bufs=6,
                                                space="PSUM"))

        dram = ctx.enter_context(tc.tile_pool(name="dram", bufs=1,
                                              space="DRAM"))

        ident = sb_const.tile([128, 128], F32)
        nc.sync.dma_start(out=ident[:], in_=id_d)

        # internal DRAM buffers
        acts_dr = [dram.tile([FC, B], BF16, name=f"acts{i}") for i in range(L)]
        pbf_dr = [dram.tile([B, FC], F32, name=f"pbf{i}") for i in range(L)]
        a2a_dr = [dram.tile([NCORE, BSH, FC], F32, name=f"a2a{i}")
                  for i in range(L)]
        tin_dr = [dram.tile([1, BSH], F32, name=f"tin{i}") for i in range(L)]
        tout_dr = [dram.tile([1, B], F32, name=f"tout{i}", addr_space="Shared")
                   for i in range(L)]
        rsin_dr = [dram.tile([OD, B], F32, name=f"rsin{j}") for j in range(L)]
        rsout_dr = [dram.tile([OSH, B], F32, name=f"rsout{j}") for j in range(L)]

        def encode_layer(i):
            # W_enc[i] as 6 k-tiles of [128, 512]
            wts = []
            for k in range(KD):
                wt = sb_we.tile([128, FC], F32, name=f"we_{i}_{k}", tag="we")
                nc.sync.dma_start(out=wt[:], in_=we_d[i, k * 128:(k + 1) * 128, :])
                wts.append(wt)
            bts = []
            for f in range(NFT):
                bt = sb_be.tile([128, 1], F32, name=f"be_{i}_{f}", tag="be")
                nc.sync.dma_start(out=bt[:],
                                  in_=be_d[i, f * 128:(f + 1) * 128][:, None])
                bts.append(bt)
            pre = [sb_pre.tile([128, B], F32, name=f"pre_{i}_{f}", tag="pre")
                   for f in range(NFT)]
            for b in range(NB):
                xts = []
                for k in range(KD):
                    xt = sb_x.tile([128, BCH], F32, name=f"x_{i}_{b}_{k}",
                                   tag="xt")
                    nc.sync.dma_start(
                        out=xt[:],
                        in_=x_d[i, k * 128:(k + 1) * 128,
                                b * BCH:(b + 1) * BCH])
                    xts.append(xt)
                for f in range(NFT):
                    ps = ps_enc.tile([128, BCH], F32, name=f"eps_{i}_{b}_{f}",
                                     tag="eps")
                    for k in range(KD):
                        nc.tensor.matmul(ps[:],
                                         wts[k][:, f * 128:(f + 1) * 128],
                                         xts[k][:],
                                         start=(k == 0), stop=(k == KD - 1))
                    nc.scalar.activation(pre[f][:, b * BCH:(b + 1) * BCH],
                                         ps[:], RELU, bias=bts[f][:], scale=1.0)
            return pre

        def topk_layer(i, pre):
            # transpose pre^T -> [b, f] staging tiles, ship to DRAM for A2A
            for bt in range(NBT):
                bft = sb_bf.tile([128, FC], F32, name=f"bf_{i}_{bt}", tag="bf")
                tps = ps_enc.tile([128, FC], F32, name=f"tps_{i}_{bt}",
                                  tag="eps")
                for f in range(NFT):
                    nc.tensor.transpose(
                        tps[:, f * 128:(f + 1) * 128],
                        pre[f][:, bt * 128:(bt + 1) * 128], ident[:])
                nc.scalar.activation(bft[:], tps[:],
                                     mybir.ActivationFunctionType.Copy)
                nc.sync.dma_start(out=pbf_dr[i][bt * 128:(bt + 1) * 128, :],
                                  in_=bft[:])
            # exchange pre slices: core c gets full 4096-wide rows for its shard
            if not sim:
                nc.gpsimd.collective_compute(
                    "AllToAll", BYP, replica_groups=RG,
                    ins=[pbf_dr[i][:].opt()], outs=[a2a_dr[i][:].opt()])
            sel_src = (pbf_dr[i][:].rearrange("(r p) k -> r p k", r=NCORE)
                       if sim else a2a_dr[i][:])
            # exact global top-64 threshold for the 256-row shard
            for bt in range(BSH // 128):
                st = sb_sel.tile([128, NCORE * FC], F32, name=f"st_{i}_{bt}",
                                 tag="st")
                src = sel_src[:, bt * 128:(bt + 1) * 128, :].rearrange(
                    "r p k -> p r k")
                nc.sync.dma_start(out=st[:].rearrange("p (r k) -> p r k",
                                                      r=NCORE), in_=src)
                sc = sb_t.tile([128, TOPK], F32, name=f"sc_{i}_{bt}", tag="sc")
                for r in range(topk_rounds):
                    nc.vector.max(sc[:, r * 8:(r + 1) * 8], st[:])
                    if r < 7:
                        nc.vector.match_replace(st[:], sc[:, r * 8:(r + 1) * 8],
                                                st[:], NEG)
                nc.sync.dma_start(out=tin_dr[i][0, bt * 128:(bt + 1) * 128],
                                  in_=sc[:, 63:64])
            if not sim:
                nc.gpsimd.collective_compute(
                    "AllGather", BYP, replica_groups=RG,
                    ins=[tin_dr[i][:].opt()], outs=[tout_dr[i][:].opt()])
            # mask pre^T in place with broadcast thresholds, store acts^T
            tb = sb_tb.tile([128, B], F32, name=f"tb_{i}", tag="tb")
            nc.sync.dma_start(out=tb[:],
                              in_=tout_dr[i][0:1, :].to_broadcast([128, B]))
            for f in range(NFT):
                mk = sb_msk.tile([128, B], F32, name=f"mk_{i}_{f}", tag="mk")
                nc.vector.tensor_tensor(mk[:], pre[f][:], tb[:], GE)
                ab = sb_msk.tile([128, B], BF16, name=f"ab_{i}_{f}", tag="ab")
                nc.vector.tensor_tensor(ab[:], pre[f][:], mk[:], MUL)
                nc.sync.dma_start(out=acts_dr[i][f * 128:(f + 1) * 128, :],
                                  in_=ab[:])

        def decode_layer(j):
            # recon^T[j][o,b] = sum_{i<=j} W_dec[i,j]^T @ acts^T[i]
            for b in range(NB):
                pss = [ps_dec.tile([128, BCH], F32, name=f"dps_{j}_{b}_{o}",
                                   tag="dps") for o in range(NOT)]
                first = True
                for i in range(j + 1):
                    p = PAIRS.index((i, j))
                    at = sb_ad.tile([128, NFT * BCH], BF16,
                                    name=f"at_{j}_{b}_{i}", tag="at")
                    nc.gpsimd.dma_start(
                        out=at[:].rearrange("p (f c) -> p f c", f=NFT),
                        in_=acts_dr[i][:].rearrange(
                            "(f p) c -> p f c", f=NFT)[:, :,
                                                       b * BCH:(b + 1) * BCH])
                    wt = sb_wd.tile([128, NFT * OD], BF16,
                                    name=f"wt_{j}_{b}_{i}", tag="wt")
                    nc.sync.dma_start(out=wt[:], in_=wd_d[p])
                    for f in range(NFT):
                        last = (i == j and f == NFT - 1)
                        for o in range(NOT):
                            nc.tensor.matmul(
                                pss[o][:],
                                wt[:, f * OD + o * 128:f * OD + (o + 1) * 128],
                                at[:, f * BCH:(f + 1) * BCH],
                                start=first, stop=last)
                        first = False
                for o in range(NOT):
                    ev = sb_ev.tile([128, BCH], F32, name=f"ev_{j}_{b}_{o}",
                                    tag="ev")
                    nc.scalar.activation(ev[:], pss[o][:],
                                         mybir.ActivationFunctionType.Copy)
                    nc.sync.dma_start(
                        out=rsin_dr[j][o * 128:(o + 1) * 128,
                                       b * BCH:(b + 1) * BCH],
                        in_=ev[:])
            if not sim:
                nc.gpsimd.collective_compute(
                    "ReduceScatter", ADD, replica_groups=RG,
                    ins=[rsin_dr[j][:].opt()], outs=[rsout_dr[j][:].opt()])
            # bias + emit this core's o-shard
            ot = sb_out.tile([OSH, B], F32, name=f"ot_{j}", tag="ot")
            nc.sync.dma_start(out=ot[:], in_=(rsin_dr[j][0:OSH, :] if sim
                                              else rsout_dr[j][:]))
            bdt = sb_bd.tile([OSH, 1], F32, name=f"bd_{j}", tag="bd")
            nc.sync.dma_start(out=bdt[:], in_=bd_d[j, :][:, None])
            nc.vector.tensor_scalar(ot[:], ot[:], bdt[:], None, ADD)
            nc.sync.dma_start(out=out_d[j], in_=ot[:])

        for lyr in range(L):
            if not no_encode:
                pre = encode_layer(lyr)
                topk_layer(lyr, pre)
            if not no_decode:
                decode_layer(lyr)

    nc.compile()
    return nc


_NC_CACHE = None


def kernel(**inputs) -> np.ndarray:
    global _NC_CACHE
    from concourse.bass_utils import run_bass_kernel_spmd

    import ml_dtypes

    x = np.ascontiguousarray(inputs["inputs"])          # [L, B, D]
    W_enc = np.ascontiguousarray(inputs["W_enc"])       # [L, D, FD]
    b_enc = np.ascontiguousarray(inputs["b_enc"])       # [L, FD]
    W_dec = np.ascontiguousarray(inputs["W_dec"])       # [L, L, FD, OD]
    b_dec = np.ascontiguousarray(inputs["b_dec"])       # [L, OD]

    x_t = np.ascontiguousarray(x.transpose(0, 2, 1))    # [L, D, B]
    ident = np.eye(128, dtype=np.float32)

    in_maps = []
    for c in range(NCORE):
        fs = slice(c * FC, (c + 1) * FC)
        wd = np.stack([W_dec[i, j, fs, :] for (i, j) in PAIRS])
        wd = np.ascontiguousarray(
            wd.reshape(len(PAIRS), 4, 128, OD).transpose(0, 2, 1, 3)
              .reshape(len(PAIRS), 128, 4 * OD)).astype(ml_dtypes.bfloat16)
        in_maps.append({
            "x_t": x_t,
            "w_enc_sl": np.ascontiguousarray(W_enc[:, :, fs]),
            "b_enc_sl": np.ascontiguousarray(b_enc[:, fs]),
            "w_dec_sl": wd,
            "b_dec_sh": np.ascontiguousarray(
                b_dec[:, c * OSH:(c + 1) * OSH]),
            "ident": ident,
        })

    if _NC_CACHE is None:
        _NC_CACHE = _build_nc()
    nc = _NC_CACHE

    trace = os.environ.get("KERNEL_TRACE", "0") == "1"
    try:
        res = run_bass_kernel_spmd(nc, in_maps, core_ids=list(range(NCORE)),
                                   trace=trace)
    except ModuleNotFoundError:
        # axon NTFF profiling hook unavailable in this container
        res = run_bass_kernel_spmd(nc, in_maps, core_ids=list(range(NCORE)))
    if res.exec_time_ns is not None:
        print(f"HW exec time: {res.exec_time_ns} ns")
        if res.instructions_and_trace is not None:
            print("trace:", res.instructions_and_trace[1])

    # unshard: concat o-shards of recon^T, then transpose to [L, B, OD]
    full_t = np.concatenate([res.results[c]["out_shard"]
                             for c in range(NCORE)], axis=1)  # [L, OD, B]
    return np.ascontiguousarray(full_t.transpose(0, 2, 1))



# revision 20
# speedup vs baseline: 1.1390x; 1.1390x over previous
"""CrossLayerTranscoder Trainium2 kernel, 8-core feature-parallel.

Sharding: dict dim (4096) split 512/core. Encode computes pre^T[f,b] slices
in fp32 (PE), relu+bias fused in the PSUM evacuation (ACT). Per-layer exact
global top-64 threshold via two-stage select: PE-transpose pre^T into [b,f]
tiles kept in SBUF, 4-round max8+match_replace gives each core's local
top-32 per row (P[any 512-slice holds >32 of the global top-64] ~ 5e-13),
AllToAll ships only the 32 candidate values per (row, core); one 8-round
select over the 256 gathered candidates per row yields the exact 64th-largest
threshold. AllGather thresholds, mask pre^T in [f,b] space, and split the
masked acts into an fp8 error-feedback pair Ah + Al (Al = acts - Ah).
Triangular decode recon^T[j] = sum_{i<=j} W_dec[i,j]^T acts^T runs on the PE
in fp8e4 DoubleRow perf mode (0.25 cyc/row per k-tile) with a 3-pass
error-feedback product Ah Wh + Al Wh + Ah Wl (W_dec split host-side into
Wh + Wl fp8 pairs, x64 scaled; PSUM accumulates all passes of the full
i-chain, evacuated with scale 1/64), then per-j ReduceScatter of the 8
partial sums; each core returns its 96-row o-shard of recon^T and the host
concatenates and transposes.

Schedule: the per-layer emission order is encode(j), select(j), mask(j-1),
decode(j-1) so that (a) the PE runs layer j-1's decode while the DVE runs
layer j's select, and (b) no engine queue blocks head-of-line on the select
round-trip (mask/quantize of a layer is emitted only after its thresholds
are already available).  DMA issue is spread across the SP queue (loads),
the ACT HWDGE queue (PSUM-evac stores, act stores), and the gpsimd SWDGE
queue (decode act loads).
"""
import os
from contextlib import ExitStack

import numpy as np

L = 12          # layers
B = 2048        # batch rows
D = 768         # d_in
FD = 4096       # dict size
OD = 768        # d_out
TOPK = 64
NCORE = 8
FC = FD // NCORE            # 512 local features
BCH = 512                   # matmul moving-dim chunk
NB = B // BCH               # 4
NBT = B // 128              # 16 topk row tiles
KD = D // 128               # 6 encode k-tiles
NFT = FC // 128             # 4 local f-tiles
NOT = OD // 128             # 6 o-tiles
OSH = OD // NCORE           # 96 output rows per core
BSH = B // NCORE            # 256 threshold rows per core
NEG = -3.0e38
NC1 = 32                    # stage-1 candidates per (row, core)
R1 = NC1 // 8               # 4 stage-1 select rounds
WSC = 64.0                  # host-side W_dec scale (fp8 subnormal avoidance)
PAIRS = [(i, j) for j in range(L) for i in range(j + 1)]   # 78, j-major


def _build_nc(sim=False, no_decode=False, topk_rounds=8, no_encode=False):
    """sim=True: single-core, collectives stripped (TimelineSim timing)."""
    import concourse.bacc as bacc
    import concourse.mybir as mybir
    import concourse.tile as tile

    F32 = mybir.dt.float32
    FP8 = mybir.dt.float8e4
    DR = mybir.MatmulPerfMode.DoubleRow
    RELU = mybir.ActivationFunctionType.Relu
    COPY = mybir.ActivationFunctionType.Copy
    GE = mybir.AluOpType.is_ge
    MUL = mybir.AluOpType.mult
    ADD = mybir.AluOpType.add
    SUB = mybir.AluOpType.subtract
    BYP = mybir.AluOpType.bypass
    RG = [list(range(NCORE))]

    nc = bacc.Bacc("TRN2", target_bir_lowering=False, debug=False,
                   num_devices=1 if sim else NCORE)

    x_d = nc.dram_tensor("x_t", [L, D, B], F32, kind="ExternalInput").ap()
    we_d = nc.dram_tensor("w_enc_sl", [L, D, FC], F32, kind="ExternalInput").ap()
    be_d = nc.dram_tensor("b_enc_sl", [L, FC], F32, kind="ExternalInput").ap()
    # interleaved Wh/Wl fp8 pair per (i,j) pair: [pair, kp, {h,l}, f, od]
    whl_d = nc.dram_tensor("w_dec_hl", [len(PAIRS), 128, 2 * NFT * OD], FP8,
                           kind="ExternalInput").ap()
    bd_d = nc.dram_tensor("b_dec_sh", [L, OSH], F32, kind="ExternalInput").ap()
    id_d = nc.dram_tensor("ident", [128, 128], F32, kind="ExternalInput").ap()
    out_d = nc.dram_tensor("out_shard", [L, OSH, B], F32,
                           kind="ExternalOutput").ap()

    with tile.TileContext(nc) as tc, ExitStack() as ctx:
        sb_const = ctx.enter_context(tc.tile_pool(name="const", bufs=1))
        sb_x = ctx.enter_context(tc.tile_pool(name="xt", bufs=6))
        sb_we = ctx.enter_context(tc.tile_pool(name="we", bufs=6))
        sb_be = ctx.enter_context(tc.tile_pool(name="be", bufs=8))
        sb_pre = ctx.enter_context(tc.tile_pool(name="pre", bufs=5))
        sb_bf = ctx.enter_context(tc.tile_pool(name="prebf", bufs=2))
        sb_c1 = ctx.enter_context(tc.tile_pool(name="cand", bufs=4))
        sb_sel = ctx.enter_context(tc.tile_pool(name="sel", bufs=2))
        sb_t = ctx.enter_context(tc.tile_pool(name="tsel", bufs=4))
        sb_tb = ctx.enter_context(tc.tile_pool(name="tbc", bufs=2))
        sb_msk = ctx.enter_context(tc.tile_pool(name="msk", bufs=1))
        sb_q = ctx.enter_context(tc.tile_pool(name="quant", bufs=2))
        sb_wd = ctx.enter_context(tc.tile_pool(name="wd", bufs=12))
        sb_ad = ctx.enter_context(tc.tile_pool(name="ad", bufs=3))
        sb_ev = ctx.enter_context(tc.tile_pool(name="ev", bufs=2))
        sb_out = ctx.enter_context(tc.tile_pool(name="outp", bufs=1))
        sb_bd = ctx.enter_context(tc.tile_pool(name="bdec", bufs=2))

        ps_enc = ctx.enter_context(tc.tile_pool(name="psenc", bufs=2,
                                                space="PSUM"))
        ps_dec = ctx.enter_context(tc.tile_pool(name="psdec", bufs=6,
                                                space="PSUM"))

        dram = ctx.enter_context(tc.tile_pool(name="dram", bufs=1,
                                              space="DRAM"))

        ident = sb_const.tile([128, 128], F32)
        nc.sync.dma_start(out=ident[:], in_=id_d)

        # internal DRAM buffers; acts stored as interleaved Ah/Al fp8 pair
        hl_dr = [dram.tile([2 * FC, B], FP8, name=f"ahl{i}") for i in range(L)]
        cand_dr = [dram.tile([B, NC1], F32, name=f"cand{i}") for i in range(L)]
        c2a_dr = [dram.tile([NCORE, BSH, NC1], F32, name=f"c2a{i}")
                  for i in range(L)]
        tin_dr = [dram.tile([1, BSH], F32, name=f"tin{i}") for i in range(L)]
        tout_dr = [dram.tile([1, B], F32, name=f"tout{i}", addr_space="Shared")
                   for i in range(L)]
        rsin_dr = [dram.tile([OD, B], F32, name=f"rsin{j}") for j in range(L)]
        rsout_dr = [dram.tile([OSH, B], F32, name=f"rsout{j}") for j in range(L)]

        def encode_layer(i):
            # W_enc[i] as 6 k-tiles of [128, 512]
            wts = []
            for k in range(KD):
                wt = sb_we.tile([128, FC], F32, name=f"we_{i}_{k}", tag="we")
                nc.sync.dma_start(out=wt[:], in_=we_d[i, k * 128:(k + 1) * 128, :])
                wts.append(wt)
            bts = []
            for f in range(NFT):
                bt = sb_be.tile([128, 1], F32, name=f"be_{i}_{f}", tag="be")
                nc.sync.dma_start(out=bt[:],
                                  in_=be_d[i, f * 128:(f + 1) * 128][:, None])
                bts.append(bt)
            pre = [sb_pre.tile([128, B], F32, name=f"pre_{i}_{f}", tag="pre")
                   for f in range(NFT)]
            for b in range(NB):
                xts = []
                for k in range(KD):
                    xt = sb_x.tile([128, BCH], F32, name=f"x_{i}_{b}_{k}",
                                   tag="xt")
                    nc.sync.dma_start(
                        out=xt[:],
                        in_=x_d[i, k * 128:(k + 1) * 128,
                                b * BCH:(b + 1) * BCH])
                    xts.append(xt)
                for f in range(NFT):
                    ps = ps_enc.tile([128, BCH], F32, name=f"eps_{i}_{b}_{f}",
                                     tag="eps")
                    for k in range(KD):
                        nc.tensor.matmul(ps[:],
                                         wts[k][:, f * 128:(f + 1) * 128],
                                         xts[k][:],
                                         start=(k == 0), stop=(k == KD - 1))
                    nc.scalar.activation(pre[f][:, b * BCH:(b + 1) * BCH],
                                         ps[:], RELU, bias=bts[f][:], scale=1.0)
            return pre

        def select_layer(i, pre):
            # stage 1: transpose pre^T -> [b, f] tiles in SBUF; local top-32
            # per row via 4 rounds of max8 + match_replace; ship candidates.
            for bt in range(NBT):
                tps = ps_enc.tile([128, FC], F32, name=f"tps_{i}_{bt}",
                                  tag="eps")
                for f in range(NFT):
                    nc.tensor.transpose(
                        tps[:, f * 128:(f + 1) * 128],
                        pre[f][:, bt * 128:(bt + 1) * 128], ident[:])
                bft = sb_bf.tile([128, FC], F32, name=f"bf_{i}_{bt}", tag="bf")
                nc.scalar.activation(bft[:], tps[:], COPY)
                sc1 = sb_c1.tile([128, NC1], F32, name=f"c1_{i}_{bt}",
                                 tag="c1")
                for r in range(R1):
                    nc.vector.max(sc1[:, r * 8:(r + 1) * 8], bft[:])
                    if r < R1 - 1:
                        nc.vector.match_replace(bft[:], sc1[:, r * 8:(r + 1) * 8],
                                                bft[:], NEG)
                nc.sync.dma_start(out=cand_dr[i][bt * 128:(bt + 1) * 128, :],
                                  in_=sc1[:])
            # exchange candidates: core c gets all 8 cores' top-32 for its rows
            if not sim:
                nc.gpsimd.collective_compute(
                    "AllToAll", BYP, replica_groups=RG,
                    ins=[cand_dr[i][:].opt()], outs=[c2a_dr[i][:].opt()])
            sel_src = (cand_dr[i][:].rearrange("(r p) k -> r p k", r=NCORE)
                       if sim else c2a_dr[i][:])
            # stage 2: exact global top-64 threshold from the 256 candidates
            for bt in range(BSH // 128):
                st = sb_sel.tile([128, NCORE * NC1], F32, name=f"st_{i}_{bt}",
                                 tag="st")
                src = sel_src[:, bt * 128:(bt + 1) * 128, :].rearrange(
                    "r p k -> p r k")
                nc.sync.dma_start(out=st[:].rearrange("p (r k) -> p r k",
                                                      r=NCORE), in_=src)
                sc = sb_t.tile([128, TOPK], F32, name=f"sc_{i}_{bt}", tag="sc")
                for r in range(topk_rounds):
                    nc.vector.max(sc[:, r * 8:(r + 1) * 8], st[:])
                    if r < 7:
                        nc.vector.match_replace(st[:], sc[:, r * 8:(r + 1) * 8],
                                                st[:], NEG)
                nc.sync.dma_start(out=tin_dr[i][0, bt * 128:(bt + 1) * 128],
                                  in_=sc[:, 63:64])
            if not sim:
                nc.gpsimd.collective_compute(
                    "AllGather", BYP, replica_groups=RG,
                    ins=[tin_dr[i][:].opt()], outs=[tout_dr[i][:].opt()])
            tb = sb_tb.tile([128, B], F32, name=f"tb_{i}", tag="tb")
            nc.sync.dma_start(out=tb[:],
                              in_=tout_dr[i][0:1, :].to_broadcast([128, B]))
            return tb

        def mask_layer(i, pre, tb):
            # mask pre^T with broadcast thresholds; split acts into fp8
            # error-feedback pair Ah + Al stored interleaved in hl_dr
            for f in range(NFT):
                mk = sb_msk.tile([128, B], F32, name=f"mk_{i}_{f}", tag="mk")
                nc.vector.tensor_tensor(mk[:], pre[f][:], tb[:], GE)
                nc.vector.tensor_tensor(mk[:], pre[f][:], mk[:], MUL)
                ah = sb_q.tile([128, B], FP8, name=f"ah_{i}_{f}", tag="ah")
                nc.vector.tensor_scalar(ah[:], mk[:], 1.0, None, MUL)
                al = sb_q.tile([128, B], FP8, name=f"al_{i}_{f}", tag="al")
                nc.vector.tensor_tensor(al[:], mk[:], ah[:], SUB)
                nc.sync.dma_start(out=hl_dr[i][f * 128:(f + 1) * 128, :],
                                  in_=ah[:])
                nc.sync.dma_start(out=hl_dr[i][FC + f * 128:FC + (f + 1) * 128, :],
                                  in_=al[:])

        def load_wd_layer(j):
            wds = []
            for i in range(j + 1):
                p = PAIRS.index((i, j))
                wd = sb_wd.tile([128, 2, NFT, OD], FP8,
                                name=f"wd_{j}_{i}", tag="wd")
                nc.sync.dma_start(out=wd[:], in_=whl_d[p].rearrange(
                    "p (t f o) -> p t f o", t=2, f=NFT))
                wds.append(wd)
            return wds

        def decode_layer(j, wds):
            # recon^T[j][o,b] = sum_{i<=j} W_dec[i,j]^T @ acts^T[i], 3-pass
            # fp8 error-feedback: Ah Wh + Al Wh + Ah Wl (all x64, evac /64)
            for b in range(NB):
                pss = [ps_dec.tile([128, BCH], F32, name=f"dps_{j}_{b}_{o}",
                                   tag="dps") for o in range(NOT)]
                first = True
                for i in range(j + 1):
                    at = sb_ad.tile([128, 2, NFT, BCH], FP8,
                                    name=f"at_{j}_{b}_{i}", tag="at")
                    nc.gpsimd.dma_start(
                        out=at[:],
                        in_=hl_dr[i][:].rearrange(
                            "(t f p) c -> p t f c", t=2,
                            f=NFT)[:, :, :, b * BCH:(b + 1) * BCH])
                    wdt = wds[i]
                    for f2 in range(0, NFT, 2):
                        last_f2 = (i == j and f2 == NFT - 2)
                        for o in range(NOT):
                            wh_s = wdt[:, 0, f2:f2 + 2,
                                        o * 128:(o + 1) * 128]
                            wl_s = wdt[:, 1, f2:f2 + 2,
                                        o * 128:(o + 1) * 128]
                            ah_s = at[:, 0, f2:f2 + 2, :]
                            al_s = at[:, 1, f2:f2 + 2, :]
                            nc.tensor.matmul(pss[o][:], wh_s, ah_s,
                                             start=first, stop=False,
                                             perf_mode=DR)
                            nc.tensor.matmul(pss[o][:], wh_s, al_s,
                                             start=False, stop=False,
                                             perf_mode=DR)
                            nc.tensor.matmul(pss[o][:], wl_s, ah_s,
                                             start=False, stop=last_f2,
                                             perf_mode=DR)
                        first = False
                for o in range(NOT):
                    ev = sb_ev.tile([128, BCH], F32, name=f"ev_{j}_{b}_{o}",
                                    tag="ev")
                    nc.scalar.activation(ev[:], pss[o][:], COPY,
                                         scale=1.0 / WSC)
                    nc.scalar.dma_start(
                        out=rsin_dr[j][o * 128:(o + 1) * 128,
                                       b * BCH:(b + 1) * BCH],
                        in_=ev[:])
            if not sim:
                nc.gpsimd.collective_compute(
                    "ReduceScatter", ADD, replica_groups=RG,
                    ins=[rsin_dr[j][:].opt()], outs=[rsout_dr[j][:].opt()])
            # bias + emit this core's o-shard (two B-halves to bound SBUF);
            # the whole chain lives on the ACT queue so the DVE never
            # head-of-line blocks the next layer's select on it
            bdt = sb_bd.tile([OSH, 1], F32, name=f"bd_{j}", tag="bd")
            nc.scalar.dma_start(out=bdt[:], in_=bd_d[j, :][:, None])
            for h in range(2):
                hb = B // 2
                src = rsin_dr[j][0:OSH, :] if sim else rsout_dr[j][:]
                ot = sb_out.tile([OSH, hb], F32, name=f"ot_{j}_{h}", tag="ot")
                nc.scalar.dma_start(out=ot[:], in_=src[:, h * hb:(h + 1) * hb])
                nc.scalar.activation(ot[:], ot[:],
                                     mybir.ActivationFunctionType.Identity,
                                     bias=bdt[:])
                nc.scalar.dma_start(out=out_d[j][:, h * hb:(h + 1) * hb],
                                    in_=ot[:])

        # software pipeline: mask+decode of layer j-1 are emitted after
        # select(j) so the PE chews on ready decode work while DVE runs the
        # layer-j select, and no queue head-of-line-blocks on thresholds.
        for lyr in range(L):
            if not no_encode:
                pre = encode_layer(lyr)
                wds = (load_wd_layer(lyr - 1)
                       if lyr > 0 and not no_decode else None)
                tb = select_layer(lyr, pre)
                mask_layer(lyr, pre, tb)
                if wds is not None:
                    decode_layer(lyr - 1, wds)
            elif not no_decode:
                decode_layer(lyr, load_wd_layer(lyr))
        if not no_encode and not no_decode:
            decode_layer(L - 1, load_wd_layer(L - 1))

    nc.compile()
    return nc


_NC_CACHE = None


def kernel(**inputs) -> np.ndarray:
    global _NC_CACHE
    from concourse.bass_utils import run_bass_kernel_spmd

    import ml_dtypes

    x = np.ascontiguousarray(inputs["inputs"])          # [L, B, D]
    W_enc = np.ascontiguousarray(inputs["W_enc"])       # [L, D, FD]
    b_enc = np.ascontiguousarray(inputs["b_enc"])       # [L, FD]
    W_dec = np.ascontiguousarray(inputs["W_dec"])       # [L, L, FD, OD]
    b_dec = np.ascontiguousarray(inputs["b_dec"])       # [L, OD]

    x_t = np.ascontiguousarray(x.transpose(0, 2, 1))    # [L, D, B]
    ident = np.eye(128, dtype=np.float32)

    in_maps = []
    for c in range(NCORE):
        fs = slice(c * FC, (c + 1) * FC)
        wd = np.stack([W_dec[i, j, fs, :] for (i, j) in PAIRS]) * WSC
        # [pair, f, kp, od] -> [pair, kp, f, od]
        wd = wd.reshape(len(PAIRS), NFT, 128, OD).transpose(0, 2, 1, 3)
        wd = np.ascontiguousarray(wd).astype(np.float32)
        wh = wd.astype(ml_dtypes.float8_e4m3)
        wl = (wd - wh.astype(np.float32)).astype(ml_dtypes.float8_e4m3)
        whl = np.stack([wh, wl], axis=2)        # [pair, kp, {h,l}, f, od]
        whl = np.ascontiguousarray(
            whl.reshape(len(PAIRS), 128, 2 * NFT * OD))
        in_maps.append({
            "x_t": x_t,
            "w_enc_sl": np.ascontiguousarray(W_enc[:, :, fs]),
            "b_enc_sl": np.ascontiguousarray(b_enc[:, fs]),
            "w_dec_hl": whl,
            "b_dec_sh": np.ascontiguousarray(
                b_dec[:, c * OSH:(c + 1) * OSH]),
            "ident": ident,
        })

    if _NC_CACHE is None:
        _NC_CACHE = _build_nc()
    nc = _NC_CACHE

    trace = os.environ.get("KERNEL_TRACE", "0") == "1"
    try:
        res = run_bass_kernel_spmd(nc, in_maps, core_ids=list(range(NCORE)),
                                   trace=trace)
    except ModuleNotFoundError:
        # axon NTFF profiling hook unavailable in this container
        res = run_bass_kernel_spmd(nc, in_maps, core_ids=list(range(NCORE)))
    if res.exec_time_ns is not None:
        print(f"HW exec time: {res.exec_time_ns} ns")
        if res.instructions_and_trace is not None:
            print("trace:", res.instructions_and_trace[1])

    # unshard: concat o-shards of recon^T, then transpose to [L, B, OD]
    full_t = np.concatenate([res.results[c]["out_shard"]
                             for c in range(NCORE)], axis=1)  # [L, OD, B]
    return np.ascontiguousarray(full_t.transpose(0, 2, 1))


# revision 27
# speedup vs baseline: 1.1604x; 1.0187x over previous
"""CrossLayerTranscoder Trainium2 kernel, 8-core feature-parallel.

Sharding: dict dim (4096) split 512/core. Encode computes pre^T[f,b] slices
in fp32 (PE), relu+bias fused in the PSUM evacuation (ACT). Per-layer exact
global top-64 threshold via two-stage select: PE-transpose pre^T into [b,f]
tiles kept in SBUF, 4-round max8+match_replace gives each core's local
top-32 per row (P[any 512-slice holds >32 of the global top-64] ~ 5e-13),
AllToAll ships only the 32 candidate values per (row, core); one 8-round
select over the 256 gathered candidates per row yields the exact 64th-largest
threshold. AllGather thresholds, mask pre^T in [f,b] space, and split the
masked acts into an fp8 error-feedback pair Ah + Al (Al = acts - Ah).
Triangular decode recon^T[j] = sum_{i<=j} W_dec[i,j]^T acts^T runs on the PE
in fp8e4 DoubleRow perf mode (0.25 cyc/row per k-tile) with a 3-pass
error-feedback product Ah Wh + Al Wh + Ah Wl (W_dec split host-side into
Wh + Wl fp8 pairs, x64 scaled; PSUM accumulates all passes of the full
i-chain, evacuated with scale 1/64), then per-j ReduceScatter of the 8
partial sums; each core returns its 96-row o-shard of recon^T and the host
concatenates and transposes.

Schedule: the per-layer emission order is encode(j), select(j), mask(j-1),
decode(j-1) so that (a) the PE runs layer j-1's decode while the DVE runs
layer j's select, and (b) no engine queue blocks head-of-line on the select
round-trip (mask/quantize of a layer is emitted only after its thresholds
are already available).  DMA issue is spread across the SP queue (loads),
the ACT HWDGE queue (PSUM-evac stores, act stores), and the gpsimd SWDGE
queue (decode act loads).
"""
import os
from contextlib import ExitStack

import numpy as np

L = 12          # layers
B = 2048        # batch rows
D = 768         # d_in
FD = 4096       # dict size
OD = 768        # d_out
TOPK = 64
NCORE = 8
FC = FD // NCORE            # 512 local features
BCH = 512                   # matmul moving-dim chunk
NB = B // BCH               # 4
NBT = B // 128              # 16 topk row tiles
KD = D // 128               # 6 encode k-tiles
NFT = FC // 128             # 4 local f-tiles
NOT = OD // 128             # 6 o-tiles
OSH = OD // NCORE           # 96 output rows per core
BSH = B // NCORE            # 256 threshold rows per core
NEG = -3.0e38
NC1 = 32                    # stage-1 candidates per (row, core)
R1 = NC1 // 8               # 4 stage-1 select rounds
WSC = 64.0                  # host-side W_dec scale (fp8 subnormal avoidance)
PAIRS = [(i, j) for j in range(L) for i in range(j + 1)]   # 78, j-major


def _build_nc(sim=False, no_decode=False, topk_rounds=8, no_encode=False):
    """sim=True: single-core, collectives stripped (TimelineSim timing)."""
    import concourse.bacc as bacc
    import concourse.mybir as mybir
    import concourse.tile as tile

    F32 = mybir.dt.float32
    FP8 = mybir.dt.float8e4
    DR = mybir.MatmulPerfMode.DoubleRow
    RELU = mybir.ActivationFunctionType.Relu
    COPY = mybir.ActivationFunctionType.Copy
    GE = mybir.AluOpType.is_ge
    MUL = mybir.AluOpType.mult
    ADD = mybir.AluOpType.add
    SUB = mybir.AluOpType.subtract
    BYP = mybir.AluOpType.bypass
    RG = [list(range(NCORE))]

    nc = bacc.Bacc("TRN2", target_bir_lowering=False, debug=False,
                   num_devices=1 if sim else NCORE)

    x_d = nc.dram_tensor("x_t", [L, D, B], F32, kind="ExternalInput").ap()
    we_d = nc.dram_tensor("w_enc_sl", [L, D, FC], F32, kind="ExternalInput").ap()
    be_d = nc.dram_tensor("b_enc_sl", [L, FC], F32, kind="ExternalInput").ap()
    # interleaved Wh/Wl fp8 pair per (i,j) pair: [pair, kp, {h,l}, f, od]
    whl_d = nc.dram_tensor("w_dec_hl", [len(PAIRS), 128, 2 * NFT * OD], FP8,
                           kind="ExternalInput").ap()
    bd_d = nc.dram_tensor("b_dec_sh", [L, OSH], F32, kind="ExternalInput").ap()
    id_d = nc.dram_tensor("ident", [128, 128], F32, kind="ExternalInput").ap()
    out_d = nc.dram_tensor("out_shard", [L, OSH, B], F32,
                           kind="ExternalOutput").ap()

    with tile.TileContext(nc) as tc, ExitStack() as ctx:
        sb_const = ctx.enter_context(tc.tile_pool(name="const", bufs=1))
        sb_x = ctx.enter_context(tc.tile_pool(name="xt", bufs=6))
        sb_we = ctx.enter_context(tc.tile_pool(name="we", bufs=6))
        sb_be = ctx.enter_context(tc.tile_pool(name="be", bufs=8))
        sb_pre = ctx.enter_context(tc.tile_pool(name="pre", bufs=5))
        sb_bf = ctx.enter_context(tc.tile_pool(name="prebf", bufs=2))
        sb_c1 = ctx.enter_context(tc.tile_pool(name="cand", bufs=4))
        sb_sel = ctx.enter_context(tc.tile_pool(name="sel", bufs=2))
        sb_t = ctx.enter_context(tc.tile_pool(name="tsel", bufs=4))
        sb_tb = ctx.enter_context(tc.tile_pool(name="tbc", bufs=2))
        sb_msk = ctx.enter_context(tc.tile_pool(name="msk", bufs=1))
        sb_q = ctx.enter_context(tc.tile_pool(name="quant", bufs=2))
        sb_wd = ctx.enter_context(tc.tile_pool(name="wd", bufs=12))
        sb_ad = ctx.enter_context(tc.tile_pool(name="ad", bufs=3))
        sb_ev = ctx.enter_context(tc.tile_pool(name="ev", bufs=2))
        sb_out = ctx.enter_context(tc.tile_pool(name="outp", bufs=1))
        sb_bd = ctx.enter_context(tc.tile_pool(name="bdec", bufs=2))

        ps_enc = ctx.enter_context(tc.tile_pool(name="psenc", bufs=2,
                                                space="PSUM"))
        ps_dec = ctx.enter_context(tc.tile_pool(name="psdec", bufs=6,
                                                space="PSUM"))

        dram = ctx.enter_context(tc.tile_pool(name="dram", bufs=1,
                                              space="DRAM"))

        ident = sb_const.tile([128, 128], F32)
        nc.sync.dma_start(out=ident[:], in_=id_d)

        # internal DRAM buffers; acts stored as interleaved Ah/Al fp8 pair
        hl_dr = [dram.tile([2 * FC, B], FP8, name=f"ahl{i}") for i in range(L)]
        cand_dr = [dram.tile([B, NC1], F32, name=f"cand{i}") for i in range(L)]
        c2a_dr = [dram.tile([NCORE, BSH, NC1], F32, name=f"c2a{i}")
                  for i in range(L)]
        tin_dr = [dram.tile([1, BSH], F32, name=f"tin{i}") for i in range(L)]
        tout_dr = [dram.tile([1, B], F32, name=f"tout{i}", addr_space="Shared")
                   for i in range(L)]
        rsin_dr = [dram.tile([OD, B], F32, name=f"rsin{j}") for j in range(L)]
        rsout_dr = [dram.tile([OSH, B], F32, name=f"rsout{j}") for j in range(L)]

        def encode_layer(i):
            # W_enc[i] as 6 k-tiles of [128, 512]
            wts = []
            for k in range(KD):
                wt = sb_we.tile([128, FC], F32, name=f"we_{i}_{k}", tag="we")
                nc.sync.dma_start(out=wt[:], in_=we_d[i, k * 128:(k + 1) * 128, :])
                wts.append(wt)
            bts = []
            for f in range(NFT):
                bt = sb_be.tile([128, 1], F32, name=f"be_{i}_{f}", tag="be")
                nc.sync.dma_start(out=bt[:],
                                  in_=be_d[i, f * 128:(f + 1) * 128][:, None])
                bts.append(bt)
            pre = [sb_pre.tile([128, B], F32, name=f"pre_{i}_{f}", tag="pre")
                   for f in range(NFT)]
            for b in range(NB):
                xts = []
                for k in range(KD):
                    xt = sb_x.tile([128, BCH], F32, name=f"x_{i}_{b}_{k}",
                                   tag="xt")
                    nc.sync.dma_start(
                        out=xt[:],
                        in_=x_d[i, k * 128:(k + 1) * 128,
                                b * BCH:(b + 1) * BCH])
                    xts.append(xt)
                for f in range(NFT):
                    ps = ps_enc.tile([128, BCH], F32, name=f"eps_{i}_{b}_{f}",
                                     tag="eps")
                    for k in range(KD):
                        nc.tensor.matmul(ps[:],
                                         wts[k][:, f * 128:(f + 1) * 128],
                                         xts[k][:],
                                         start=(k == 0), stop=(k == KD - 1))
                    nc.scalar.activation(pre[f][:, b * BCH:(b + 1) * BCH],
                                         ps[:], RELU, bias=bts[f][:], scale=1.0)
            return pre

        def select_layer(i, pre):
            # stage 1: transpose pre^T -> [b, f] tiles in SBUF; local top-32
            # per row via 4 rounds of max8 + match_replace; ship candidates.
            for bt in range(NBT):
                tps = ps_enc.tile([128, FC], F32, name=f"tps_{i}_{bt}",
                                  tag="eps")
                for f in range(NFT):
                    nc.tensor.transpose(
                        tps[:, f * 128:(f + 1) * 128],
                        pre[f][:, bt * 128:(bt + 1) * 128], ident[:])
                bft = sb_bf.tile([128, FC], F32, name=f"bf_{i}_{bt}", tag="bf")
                nc.vector.tensor_scalar(bft[:], tps[:], 1.0, None, MUL)
                sc1 = sb_c1.tile([128, NC1], F32, name=f"c1_{i}_{bt}",
                                 tag="c1")
                for r in range(R1):
                    nc.vector.max(sc1[:, r * 8:(r + 1) * 8], bft[:])
                    if r < R1 - 1:
                        nc.vector.match_replace(bft[:], sc1[:, r * 8:(r + 1) * 8],
                                                bft[:], NEG)
                nc.sync.dma_start(out=cand_dr[i][bt * 128:(bt + 1) * 128, :],
                                  in_=sc1[:])
            # exchange candidates: core c gets all 8 cores' top-32 for its rows
            if not sim:
                nc.gpsimd.collective_compute(
                    "AllToAll", BYP, replica_groups=RG,
                    ins=[cand_dr[i][:].opt()], outs=[c2a_dr[i][:].opt()])
            sel_src = (cand_dr[i][:].rearrange("(r p) k -> r p k", r=NCORE)
                       if sim else c2a_dr[i][:])
            # stage 2: exact global top-64 threshold from the 256 candidates
            for bt in range(BSH // 128):
                st = sb_sel.tile([128, NCORE * NC1], F32, name=f"st_{i}_{bt}",
                                 tag="st")
                src = sel_src[:, bt * 128:(bt + 1) * 128, :].rearrange(
                    "r p k -> p r k")
                nc.sync.dma_start(out=st[:].rearrange("p (r k) -> p r k",
                                                      r=NCORE), in_=src)
                sc = sb_t.tile([128, TOPK], F32, name=f"sc_{i}_{bt}", tag="sc")
                for r in range(topk_rounds):
                    nc.vector.max(sc[:, r * 8:(r + 1) * 8], st[:])
                    if r < 7:
                        nc.vector.match_replace(st[:], sc[:, r * 8:(r + 1) * 8],
                                                st[:], NEG)
                nc.sync.dma_start(out=tin_dr[i][0, bt * 128:(bt + 1) * 128],
                                  in_=sc[:, 63:64])
            if not sim:
                nc.gpsimd.collective_compute(
                    "AllGather", BYP, replica_groups=RG,
                    ins=[tin_dr[i][:].opt()], outs=[tout_dr[i][:].opt()])
            tb = sb_tb.tile([128, B], F32, name=f"tb_{i}", tag="tb")
            nc.sync.dma_start(out=tb[:],
                              in_=tout_dr[i][0:1, :].to_broadcast([128, B]))
            return tb

        def mask_layer(i, pre, tb):
            # mask pre^T with broadcast thresholds; split acts into fp8
            # error-feedback pair Ah + Al stored interleaved in hl_dr
            for f in range(NFT):
                mk = sb_msk.tile([128, B], F32, name=f"mk_{i}_{f}", tag="mk")
                nc.vector.tensor_tensor(mk[:], pre[f][:], tb[:], GE)
                nc.vector.tensor_tensor(mk[:], pre[f][:], mk[:], MUL)
                ah = sb_q.tile([128, B], FP8, name=f"ah_{i}_{f}", tag="ah")
                nc.vector.tensor_scalar(ah[:], mk[:], 1.0, None, MUL)
                al = sb_q.tile([128, B], FP8, name=f"al_{i}_{f}", tag="al")
                nc.vector.tensor_tensor(al[:], mk[:], ah[:], SUB)
                nc.sync.dma_start(out=hl_dr[i][f * 128:(f + 1) * 128, :],
                                  in_=ah[:])
                nc.sync.dma_start(out=hl_dr[i][FC + f * 128:FC + (f + 1) * 128, :],
                                  in_=al[:])

        def load_wd_layer(j):
            wds = []
            for i in range(j + 1):
                p = PAIRS.index((i, j))
                wd = sb_wd.tile([128, 2, NFT, OD], FP8,
                                name=f"wd_{j}_{i}", tag="wd")
                nc.sync.dma_start(out=wd[:], in_=whl_d[p].rearrange(
                    "p (t f o) -> p t f o", t=2, f=NFT))
                wds.append(wd)
            return wds

        def decode_layer(j, wds):
            # recon^T[j][o,b] = sum_{i<=j} W_dec[i,j]^T @ acts^T[i], 3-pass
            # fp8 error-feedback: Ah Wh + Al Wh + Ah Wl (all x64, evac /64)
            for b in range(NB):
                pss = [ps_dec.tile([128, BCH], F32, name=f"dps_{j}_{b}_{o}",
                                   tag="dps") for o in range(NOT)]
                first = True
                for i in range(j + 1):
                    at = sb_ad.tile([128, 2, NFT, BCH], FP8,
                                    name=f"at_{j}_{b}_{i}", tag="at")
                    nc.sync.dma_start(
                        out=at[:],
                        in_=hl_dr[i][:].rearrange(
                            "(t f p) c -> p t f c", t=2,
                            f=NFT)[:, :, :, b * BCH:(b + 1) * BCH])
                    wdt = wds[i]
                    for f2 in range(0, NFT, 2):
                        last_f2 = (i == j and f2 == NFT - 2)
                        for o in range(NOT):
                            wh_s = wdt[:, 0, f2:f2 + 2,
                                        o * 128:(o + 1) * 128]
                            wl_s = wdt[:, 1, f2:f2 + 2,
                                        o * 128:(o + 1) * 128]
                            ah_s = at[:, 0, f2:f2 + 2, :]
                            al_s = at[:, 1, f2:f2 + 2, :]
                            nc.tensor.matmul(pss[o][:], wh_s, ah_s,
                                             start=first, stop=False,
                                             perf_mode=DR)
                            nc.tensor.matmul(pss[o][:], wh_s, al_s,
                                             start=False, stop=False,
                                             perf_mode=DR)
                            nc.tensor.matmul(pss[o][:], wl_s, ah_s,
                                             start=False, stop=last_f2,
                                             perf_mode=DR)
                        first = False
                for o in range(NOT):
                    ev = sb_ev.tile([128, BCH], F32, name=f"ev_{j}_{b}_{o}",
                                    tag="ev")
                    nc.scalar.activation(ev[:], pss[o][:], COPY,
                                         scale=1.0 / WSC)
                    nc.scalar.dma_start(
                        out=rsin_dr[j][o * 128:(o + 1) * 128,
                                       b * BCH:(b + 1) * BCH],
                        in_=ev[:])
            if not sim:
                nc.gpsimd.collective_compute(
                    "ReduceScatter", ADD, replica_groups=RG,
                    ins=[rsin_dr[j][:].opt()], outs=[rsout_dr[j][:].opt()])
            # bias + emit this core's o-shard (two B-halves to bound SBUF);
            # the whole chain lives on the ACT queue so the DVE never
            # head-of-line blocks the next layer's select on it
            bdt = sb_bd.tile([OSH, 1], F32, name=f"bd_{j}", tag="bd")
            nc.scalar.dma_start(out=bdt[:], in_=bd_d[j, :][:, None])
            for h in range(2):
                hb = B // 2
                src = rsin_dr[j][0:OSH, :] if sim else rsout_dr[j][:]
                ot = sb_out.tile([OSH, hb], F32, name=f"ot_{j}_{h}", tag="ot")
                nc.scalar.dma_start(out=ot[:], in_=src[:, h * hb:(h + 1) * hb])
                nc.scalar.activation(ot[:], ot[:],
                                     mybir.ActivationFunctionType.Identity,
                                     bias=bdt[:])
                nc.scalar.dma_start(out=out_d[j][:, h * hb:(h + 1) * hb],
                                    in_=ot[:])

        # software pipeline: mask+decode of layer j-1 are emitted after
        # select(j) so the PE chews on ready decode work while DVE runs the
        # layer-j select, and no queue head-of-line-blocks on thresholds.
        for lyr in range(L):
            if not no_encode:
                pre = encode_layer(lyr)
                wds = (load_wd_layer(lyr - 1)
                       if lyr > 0 and not no_decode else None)
                tb = select_layer(lyr, pre)
                mask_layer(lyr, pre, tb)
                if wds is not None:
                    decode_layer(lyr - 1, wds)
            elif not no_decode:
                decode_layer(lyr, load_wd_layer(lyr))
        if not no_encode and not no_decode:
            decode_layer(L - 1, load_wd_layer(L - 1))

    nc.compile()
    return nc


_NC_CACHE = None


def kernel(**inputs) -> np.ndarray:
    global _NC_CACHE
    from concourse.bass_utils import run_bass_kernel_spmd

    import ml_dtypes

    x = np.ascontiguousarray(inputs["inputs"])          # [L, B, D]
    W_enc = np.ascontiguousarray(inputs["W_enc"])       # [L, D, FD]
    b_enc = np.ascontiguousarray(inputs["b_enc"])       # [L, FD]
    W_dec = np.ascontiguousarray(inputs["W_dec"])       # [L, L, FD, OD]
    b_dec = np.ascontiguousarray(inputs["b_dec"])       # [L, OD]

    x_t = np.ascontiguousarray(x.transpose(0, 2, 1))    # [L, D, B]
    ident = np.eye(128, dtype=np.float32)

    in_maps = []
    for c in range(NCORE):
        fs = slice(c * FC, (c + 1) * FC)
        wd = np.stack([W_dec[i, j, fs, :] for (i, j) in PAIRS]) * WSC
        # [pair, f, kp, od] -> [pair, kp, f, od]
        wd = wd.reshape(len(PAIRS), NFT, 128, OD).transpose(0, 2, 1, 3)
        wd = np.ascontiguousarray(wd).astype(np.float32)
        wh = wd.astype(ml_dtypes.float8_e4m3)
        wl = (wd - wh.astype(np.float32)).astype(ml_dtypes.float8_e4m3)
        whl = np.stack([wh, wl], axis=2)        # [pair, kp, {h,l}, f, od]
        whl = np.ascontiguousarray(
            whl.reshape(len(PAIRS), 128, 2 * NFT * OD))
        in_maps.append({
            "x_t": x_t,
            "w_enc_sl": np.ascontiguousarray(W_enc[:, :, fs]),
            "b_enc_sl": np.ascontiguousarray(b_enc[:, fs]),
            "w_dec_hl": whl,
            "b_dec_sh": np.ascontiguousarray(
                b_dec[:, c * OSH:(c + 1) * OSH]),
            "ident": ident,
        })

    if _NC_CACHE is None:
        _NC_CACHE = _build_nc()
    nc = _NC_CACHE

    trace = os.environ.get("KERNEL_TRACE", "0") == "1"
    try:
        res = run_bass_kernel_spmd(nc, in_maps, core_ids=list(range(NCORE)),
                                   trace=trace)
    except ModuleNotFoundError:
        # axon NTFF profiling hook unavailable in this container
        res = run_bass_kernel_spmd(nc, in_maps, core_ids=list(range(NCORE)))
    if res.exec_time_ns is not None:
        print(f"HW exec time: {res.exec_time_ns} ns")
        if res.instructions_and_trace is not None:
            print("trace:", res.instructions_and_trace[1])

    # unshard: concat o-shards of recon^T, then transpose to [L, B, OD]
    full_t = np.concatenate([res.results[c]["out_shard"]
                             for c in range(NCORE)], axis=1)  # [L, OD, B]
    return np.ascontiguousarray(full_t.transpose(0, 2, 1))


# revision 29
# speedup vs baseline: 1.1664x; 1.0052x over previous
"""CrossLayerTranscoder Trainium2 kernel, 8-core feature-parallel.

Sharding: dict dim (4096) split 512/core. Encode computes pre^T[f,b] slices
in fp32 (PE), relu+bias fused in the PSUM evacuation (ACT). Per-layer exact
global top-64 threshold via two-stage select: PE-transpose pre^T into [b,f]
tiles kept in SBUF, 4-round max8+match_replace gives each core's local
top-32 per row (P[any 512-slice holds >32 of the global top-64] ~ 5e-13),
AllToAll ships only the 32 candidate values per (row, core); one 8-round
select over the 256 gathered candidates per row yields the exact 64th-largest
threshold. AllGather thresholds, mask pre^T in [f,b] space, and split the
masked acts into an fp8 error-feedback pair Ah + Al (Al = acts - Ah).
Triangular decode recon^T[j] = sum_{i<=j} W_dec[i,j]^T acts^T runs on the PE
in fp8e4 DoubleRow perf mode (0.25 cyc/row per k-tile) with a 3-pass
error-feedback product Ah Wh + Al Wh + Ah Wl (W_dec split host-side into
Wh + Wl fp8 pairs, x64 scaled; PSUM accumulates all passes of the full
i-chain, evacuated with scale 1/64), then per-j ReduceScatter of the 8
partial sums; each core returns its 96-row o-shard of recon^T and the host
concatenates and transposes.

Schedule: the per-layer emission order is encode(j), select(j), mask(j-1),
decode(j-1) so that (a) the PE runs layer j-1's decode while the DVE runs
layer j's select, and (b) no engine queue blocks head-of-line on the select
round-trip (mask/quantize of a layer is emitted only after its thresholds
are already available).  DMA issue is spread across the SP queue (loads),
the ACT HWDGE queue (PSUM-evac stores, act stores), and the gpsimd SWDGE
queue (decode act loads).
"""
import os
from contextlib import ExitStack

import numpy as np

L = 12          # layers
B = 2048        # batch rows
D = 768         # d_in
FD = 4096       # dict size
OD = 768        # d_out
TOPK = 64
NCORE = 8
FC = FD // NCORE            # 512 local features
BCH = 512                   # matmul moving-dim chunk
NB = B // BCH               # 4
NBT = B // 128              # 16 topk row tiles
KD = D // 128               # 6 encode k-tiles
NFT = FC // 128             # 4 local f-tiles
NOT = OD // 128             # 6 o-tiles
OSH = OD // NCORE           # 96 output rows per core
BSH = B // NCORE            # 256 threshold rows per core
NEG = -3.0e38
NC1 = 32                    # stage-1 candidates per (row, core)
R1 = NC1 // 8               # 4 stage-1 select rounds
WSC = 64.0                  # host-side W_dec scale (fp8 subnormal avoidance)
PAIRS = [(i, j) for j in range(L) for i in range(j + 1)]   # 78, j-major


def _build_nc(sim=False, no_decode=False, topk_rounds=8, no_encode=False):
    """sim=True: single-core, collectives stripped (TimelineSim timing)."""
    import concourse.bacc as bacc
    import concourse.mybir as mybir
    import concourse.tile as tile

    F32 = mybir.dt.float32
    FP8 = mybir.dt.float8e4
    DR = mybir.MatmulPerfMode.DoubleRow
    RELU = mybir.ActivationFunctionType.Relu
    COPY = mybir.ActivationFunctionType.Copy
    GE = mybir.AluOpType.is_ge
    MUL = mybir.AluOpType.mult
    ADD = mybir.AluOpType.add
    SUB = mybir.AluOpType.subtract
    BYP = mybir.AluOpType.bypass
    RG = [list(range(NCORE))]

    nc = bacc.Bacc("TRN2", target_bir_lowering=False, debug=False,
                   num_devices=1 if sim else NCORE)

    x_d = nc.dram_tensor("x_t", [L, D, B], F32, kind="ExternalInput").ap()
    we_d = nc.dram_tensor("w_enc_sl", [L, D, FC], F32, kind="ExternalInput").ap()
    be_d = nc.dram_tensor("b_enc_sl", [L, FC], F32, kind="ExternalInput").ap()
    # interleaved Wh/Wl fp8 pair per (i,j) pair: [pair, kp, {h,l}, f, od]
    whl_d = nc.dram_tensor("w_dec_hl", [len(PAIRS), 128, 2 * NFT * OD], FP8,
                           kind="ExternalInput").ap()
    bd_d = nc.dram_tensor("b_dec_sh", [L, OSH], F32, kind="ExternalInput").ap()
    id_d = nc.dram_tensor("ident", [128, 128], F32, kind="ExternalInput").ap()
    out_d = nc.dram_tensor("out_shard", [L, OSH, B], F32,
                           kind="ExternalOutput").ap()

    with tile.TileContext(nc) as tc, ExitStack() as ctx:
        sb_const = ctx.enter_context(tc.tile_pool(name="const", bufs=1))
        sb_x = ctx.enter_context(tc.tile_pool(name="xt", bufs=6))
        sb_we = ctx.enter_context(tc.tile_pool(name="we", bufs=6))
        sb_be = ctx.enter_context(tc.tile_pool(name="be", bufs=8))
        sb_pre = ctx.enter_context(tc.tile_pool(name="pre", bufs=5))
        sb_bf = ctx.enter_context(tc.tile_pool(name="prebf", bufs=2))
        sb_c1 = ctx.enter_context(tc.tile_pool(name="cand", bufs=4))
        sb_sel = ctx.enter_context(tc.tile_pool(name="sel", bufs=1))
        sb_t = ctx.enter_context(tc.tile_pool(name="tsel", bufs=4))
        sb_tb = ctx.enter_context(tc.tile_pool(name="tbc", bufs=2))
        sb_msk = ctx.enter_context(tc.tile_pool(name="msk", bufs=2))
        sb_q = ctx.enter_context(tc.tile_pool(name="quant", bufs=2))
        sb_wd = ctx.enter_context(tc.tile_pool(name="wd", bufs=12))
        sb_ad = ctx.enter_context(tc.tile_pool(name="ad", bufs=3))
        sb_ev = ctx.enter_context(tc.tile_pool(name="ev", bufs=2))
        sb_out = ctx.enter_context(tc.tile_pool(name="outp", bufs=1))
        sb_bd = ctx.enter_context(tc.tile_pool(name="bdec", bufs=2))

        ps_enc = ctx.enter_context(tc.tile_pool(name="psenc", bufs=2,
                                                space="PSUM"))
        ps_dec = ctx.enter_context(tc.tile_pool(name="psdec", bufs=6,
                                                space="PSUM"))

        dram = ctx.enter_context(tc.tile_pool(name="dram", bufs=1,
                                              space="DRAM"))

        ident = sb_const.tile([128, 128], F32)
        nc.sync.dma_start(out=ident[:], in_=id_d)

        # internal DRAM buffers; acts stored as interleaved Ah/Al fp8 pair
        hl_dr = [dram.tile([2 * FC, B], FP8, name=f"ahl{i}") for i in range(L)]
        cand_dr = [dram.tile([B, NC1], F32, name=f"cand{i}") for i in range(L)]
        c2a_dr = [dram.tile([NCORE, BSH, NC1], F32, name=f"c2a{i}")
                  for i in range(L)]
        tin_dr = [dram.tile([1, BSH], F32, name=f"tin{i}") for i in range(L)]
        tout_dr = [dram.tile([1, B], F32, name=f"tout{i}", addr_space="Shared")
                   for i in range(L)]
        rsin_dr = [dram.tile([OD, B], F32, name=f"rsin{j}") for j in range(L)]
        rsout_dr = [dram.tile([OSH, B], F32, name=f"rsout{j}") for j in range(L)]

        def encode_layer(i):
            # W_enc[i] as 6 k-tiles of [128, 512]
            wts = []
            for k in range(KD):
                wt = sb_we.tile([128, FC], F32, name=f"we_{i}_{k}", tag="we")
                nc.sync.dma_start(out=wt[:], in_=we_d[i, k * 128:(k + 1) * 128, :])
                wts.append(wt)
            bts = []
            for f in range(NFT):
                bt = sb_be.tile([128, 1], F32, name=f"be_{i}_{f}", tag="be")
                nc.sync.dma_start(out=bt[:],
                                  in_=be_d[i, f * 128:(f + 1) * 128][:, None])
                bts.append(bt)
            pre = [sb_pre.tile([128, B], F32, name=f"pre_{i}_{f}", tag="pre")
                   for f in range(NFT)]
            for b in range(NB):
                xts = []
                for k in range(KD):
                    xt = sb_x.tile([128, BCH], F32, name=f"x_{i}_{b}_{k}",
                                   tag="xt")
                    nc.sync.dma_start(
                        out=xt[:],
                        in_=x_d[i, k * 128:(k + 1) * 128,
                                b * BCH:(b + 1) * BCH])
                    xts.append(xt)
                for f in range(NFT):
                    ps = ps_enc.tile([128, BCH], F32, name=f"eps_{i}_{b}_{f}",
                                     tag="eps")
                    for k in range(KD):
                        nc.tensor.matmul(ps[:],
                                         wts[k][:, f * 128:(f + 1) * 128],
                                         xts[k][:],
                                         start=(k == 0), stop=(k == KD - 1))
                    nc.scalar.activation(pre[f][:, b * BCH:(b + 1) * BCH],
                                         ps[:], RELU, bias=bts[f][:], scale=1.0)
            return pre

        def select_layer(i, pre):
            # stage 1: transpose pre^T -> [b, f] tiles in SBUF; local top-32
            # per row via 4 rounds of max8 + match_replace; ship candidates.
            for bt in range(NBT):
                tps = ps_enc.tile([128, FC], F32, name=f"tps_{i}_{bt}",
                                  tag="eps")
                for f in range(NFT):
                    nc.tensor.transpose(
                        tps[:, f * 128:(f + 1) * 128],
                        pre[f][:, bt * 128:(bt + 1) * 128], ident[:])
                bft = sb_bf.tile([128, FC], F32, name=f"bf_{i}_{bt}", tag="bf")
                nc.vector.tensor_scalar(bft[:], tps[:], 1.0, None, MUL)
                sc1 = sb_c1.tile([128, NC1], F32, name=f"c1_{i}_{bt}",
                                 tag="c1")
                for r in range(R1):
                    nc.vector.max(sc1[:, r * 8:(r + 1) * 8], bft[:])
                    if r < R1 - 1:
                        nc.vector.match_replace(bft[:], sc1[:, r * 8:(r + 1) * 8],
                                                bft[:], NEG)
                nc.sync.dma_start(out=cand_dr[i][bt * 128:(bt + 1) * 128, :],
                                  in_=sc1[:])
            # exchange candidates: core c gets all 8 cores' top-32 for its rows
            if not sim:
                nc.gpsimd.collective_compute(
                    "AllToAll", BYP, replica_groups=RG,
                    ins=[cand_dr[i][:].opt()], outs=[c2a_dr[i][:].opt()])
            sel_src = (cand_dr[i][:].rearrange("(r p) k -> r p k", r=NCORE)
                       if sim else c2a_dr[i][:])
            # stage 2: exact global top-64 threshold from the 256 candidates
            for bt in range(BSH // 128):
                st = sb_sel.tile([128, NCORE * NC1], F32, name=f"st_{i}_{bt}",
                                 tag="st")
                src = sel_src[:, bt * 128:(bt + 1) * 128, :].rearrange(
                    "r p k -> p r k")
                nc.sync.dma_start(out=st[:].rearrange("p (r k) -> p r k",
                                                      r=NCORE), in_=src)
                sc = sb_t.tile([128, TOPK], F32, name=f"sc_{i}_{bt}", tag="sc")
                for r in range(topk_rounds):
                    nc.vector.max(sc[:, r * 8:(r + 1) * 8], st[:])
                    if r < 7:
                        nc.vector.match_replace(st[:], sc[:, r * 8:(r + 1) * 8],
                                                st[:], NEG)
                nc.sync.dma_start(out=tin_dr[i][0, bt * 128:(bt + 1) * 128],
                                  in_=sc[:, 63:64])
            if not sim:
                nc.gpsimd.collective_compute(
                    "AllGather", BYP, replica_groups=RG,
                    ins=[tin_dr[i][:].opt()], outs=[tout_dr[i][:].opt()])
            tb = sb_tb.tile([128, B], F32, name=f"tb_{i}", tag="tb")
            nc.sync.dma_start(out=tb[:],
                              in_=tout_dr[i][0:1, :].to_broadcast([128, B]))
            return tb

        def mask_layer(i, pre, tb):
            # mask pre^T with broadcast thresholds; split acts into fp8
            # error-feedback pair Ah + Al stored interleaved in hl_dr
            def mask_front(f):
                # GE+MUL ahead of the quant chain: the MUL is pre[f]'s last
                # read, so this frees the pre ring for the next encode early
                mk = sb_msk.tile([128, B], F32, name=f"mk_{i}_{f}", tag="mk")
                nc.vector.tensor_tensor(mk[:], pre[f][:], tb[:], GE)
                nc.vector.tensor_tensor(mk[:], pre[f][:], mk[:], MUL)
                return mk
            mks = {0: mask_front(0), 1: mask_front(1)}
            for f in range(NFT):
                if f + 2 < NFT:
                    mks[f + 2] = mask_front(f + 2)
                mk = mks.pop(f)
                ah = sb_q.tile([128, B], FP8, name=f"ah_{i}_{f}", tag="ah")
                nc.vector.tensor_scalar(ah[:], mk[:], 1.0, None, MUL)
                al = sb_q.tile([128, B], FP8, name=f"al_{i}_{f}", tag="al")
                nc.vector.tensor_tensor(al[:], mk[:], ah[:], SUB)
                nc.sync.dma_start(out=hl_dr[i][f * 128:(f + 1) * 128, :],
                                  in_=ah[:])
                nc.sync.dma_start(out=hl_dr[i][FC + f * 128:FC + (f + 1) * 128, :],
                                  in_=al[:])

        def load_wd_layer(j):
            wds = []
            for i in range(j + 1):
                p = PAIRS.index((i, j))
                wd = sb_wd.tile([128, 2, NFT, OD], FP8,
                                name=f"wd_{j}_{i}", tag="wd")
                nc.sync.dma_start(out=wd[:], in_=whl_d[p].rearrange(
                    "p (t f o) -> p t f o", t=2, f=NFT))
                wds.append(wd)
            return wds

        def decode_layer(j, wds):
            # recon^T[j][o,b] = sum_{i<=j} W_dec[i,j]^T @ acts^T[i], 3-pass
            # fp8 error-feedback: Ah Wh + Al Wh + Ah Wl (all x64, evac /64)
            for b in range(NB):
                pss = [ps_dec.tile([128, BCH], F32, name=f"dps_{j}_{b}_{o}",
                                   tag="dps") for o in range(NOT)]
                first = True
                for i in range(j + 1):
                    at = sb_ad.tile([128, 2, NFT, BCH], FP8,
                                    name=f"at_{j}_{b}_{i}", tag="at")
                    nc.sync.dma_start(
                        out=at[:],
                        in_=hl_dr[i][:].rearrange(
                            "(t f p) c -> p t f c", t=2,
                            f=NFT)[:, :, :, b * BCH:(b + 1) * BCH])
                    wdt = wds[i]
                    for f2 in range(0, NFT, 2):
                        last_f2 = (i == j and f2 == NFT - 2)
                        for o in range(NOT):
                            wh_s = wdt[:, 0, f2:f2 + 2,
                                        o * 128:(o + 1) * 128]
                            wl_s = wdt[:, 1, f2:f2 + 2,
                                        o * 128:(o + 1) * 128]
                            ah_s = at[:, 0, f2:f2 + 2, :]
                            al_s = at[:, 1, f2:f2 + 2, :]
                            nc.tensor.matmul(pss[o][:], wh_s, ah_s,
                                             start=first, stop=False,
                                             perf_mode=DR)
                            nc.tensor.matmul(pss[o][:], wh_s, al_s,
                                             start=False, stop=False,
                                             perf_mode=DR)
                            nc.tensor.matmul(pss[o][:], wl_s, ah_s,
                                             start=False, stop=last_f2,
                                             perf_mode=DR)
                        first = False
                for o in range(NOT):
                    ev = sb_ev.tile([128, BCH], F32, name=f"ev_{j}_{b}_{o}",
                                    tag="ev")
                    nc.scalar.activation(ev[:], pss[o][:], COPY,
                                         scale=1.0 / WSC)
                    nc.scalar.dma_start(
                        out=rsin_dr[j][o * 128:(o + 1) * 128,
                                       b * BCH:(b + 1) * BCH],
                        in_=ev[:])
            if not sim:
                nc.gpsimd.collective_compute(
                    "ReduceScatter", ADD, replica_groups=RG,
                    ins=[rsin_dr[j][:].opt()], outs=[rsout_dr[j][:].opt()])
            # bias + emit this core's o-shard (two B-halves to bound SBUF);
            # the whole chain lives on the ACT queue so the DVE never
            # head-of-line blocks the next layer's select on it
            bdt = sb_bd.tile([OSH, 1], F32, name=f"bd_{j}", tag="bd")
            nc.scalar.dma_start(out=bdt[:], in_=bd_d[j, :][:, None])
            for h in range(2):
                hb = B // 2
                src = rsin_dr[j][0:OSH, :] if sim else rsout_dr[j][:]
                ot = sb_out.tile([OSH, hb], F32, name=f"ot_{j}_{h}", tag="ot")
                nc.scalar.dma_start(out=ot[:], in_=src[:, h * hb:(h + 1) * hb])
                nc.scalar.activation(ot[:], ot[:],
                                     mybir.ActivationFunctionType.Identity,
                                     bias=bdt[:])
                nc.scalar.dma_start(out=out_d[j][:, h * hb:(h + 1) * hb],
                                    in_=ot[:])

        # software pipeline: mask+decode of layer j-1 are emitted after
        # select(j) so the PE chews on ready decode work while DVE runs the
        # layer-j select, and no queue head-of-line-blocks on thresholds.
        for lyr in range(L):
            if not no_encode:
                pre = encode_layer(lyr)
                wds = (load_wd_layer(lyr - 1)
                       if lyr > 0 and not no_decode else None)
                tb = select_layer(lyr, pre)
                mask_layer(lyr, pre, tb)
                if wds is not None:
                    decode_layer(lyr - 1, wds)
            elif not no_decode:
                decode_layer(lyr, load_wd_layer(lyr))
        if not no_encode and not no_decode:
            decode_layer(L - 1, load_wd_layer(L - 1))

    nc.compile()
    return nc


_NC_CACHE = None


def kernel(**inputs) -> np.ndarray:
    global _NC_CACHE
    from concourse.bass_utils import run_bass_kernel_spmd

    import ml_dtypes

    x = np.ascontiguousarray(inputs["inputs"])          # [L, B, D]
    W_enc = np.ascontiguousarray(inputs["W_enc"])       # [L, D, FD]
    b_enc = np.ascontiguousarray(inputs["b_enc"])       # [L, FD]
    W_dec = np.ascontiguousarray(inputs["W_dec"])       # [L, L, FD, OD]
    b_dec = np.ascontiguousarray(inputs["b_dec"])       # [L, OD]

    x_t = np.ascontiguousarray(x.transpose(0, 2, 1))    # [L, D, B]
    ident = np.eye(128, dtype=np.float32)

    in_maps = []
    for c in range(NCORE):
        fs = slice(c * FC, (c + 1) * FC)
        wd = np.stack([W_dec[i, j, fs, :] for (i, j) in PAIRS]) * WSC
        # [pair, f, kp, od] -> [pair, kp, f, od]
        wd = wd.reshape(len(PAIRS), NFT, 128, OD).transpose(0, 2, 1, 3)
        wd = np.ascontiguousarray(wd).astype(np.float32)
        wh = wd.astype(ml_dtypes.float8_e4m3)
        wl = (wd - wh.astype(np.float32)).astype(ml_dtypes.float8_e4m3)
        whl = np.stack([wh, wl], axis=2)        # [pair, kp, {h,l}, f, od]
        whl = np.ascontiguousarray(
            whl.reshape(len(PAIRS), 128, 2 * NFT * OD))
        in_maps.append({
            "x_t": x_t,
            "w_enc_sl": np.ascontiguousarray(W_enc[:, :, fs]),
            "b_enc_sl": np.ascontiguousarray(b_enc[:, fs]),
            "w_dec_hl": whl,
            "b_dec_sh": np.ascontiguousarray(
                b_dec[:, c * OSH:(c + 1) * OSH]),
            "ident": ident,
        })

    if _NC_CACHE is None:
        _NC_CACHE = _build_nc()
    nc = _NC_CACHE

    trace = os.environ.get("KERNEL_TRACE", "0") == "1"
    try:
        res = run_bass_kernel_spmd(nc, in_maps, core_ids=list(range(NCORE)),
                                   trace=trace)
    except ModuleNotFoundError:
        # axon NTFF profiling hook unavailable in this container
        res = run_bass_kernel_spmd(nc, in_maps, core_ids=list(range(NCORE)))
    if res.exec_time_ns is not None:
        print(f"HW exec time: {res.exec_time_ns} ns")
        if res.instructions_and_trace is not None:
            print("trace:", res.instructions_and_trace[1])

    # unshard: concat o-shards of recon^T, then transpose to [L, B, OD]
    full_t = np.concatenate([res.results[c]["out_shard"]
                             for c in range(NCORE)], axis=1)  # [L, OD, B]
    return np.ascontiguousarray(full_t.transpose(0, 2, 1))
